# revision 12
# baseline (speedup 1.0000x reference)
"""Trainium2 Bass kernel for nn_LocalConnectivity (diamond-ring circular stencil).

out[i,j] = sum_{d=1..5} w_d * sum_{|di|+|dj|=d} x[(i+di)%H, (j+dj)%W]

Strategy: 4x2 grid shard (1024x2048 block/core), all matmul work in fp8
DoubleRow mode (2 stacked 128-contraction streams per 256-cycle pass, 4x
the bf16 rate).  The dual-fp8 ISA requires pair strides ≡0 (mod 16B) and
even start offsets, which bans odd column shifts and same-plane ±j pairs.
Both are solved by a telescoped sigma-basis, computed on host:

  s1 = x(-1)+x(+1)  (host fp32 fold), then with V_j the vertical band of
  w_{j+|d|} taps:   sum_j V_j x(shift j)  ==
      [V0 x(0) + V2 (x(-2)+x(+2)) + V4 (x(-4)+x(+4))]            (x-family)
    + [(V1-V3+V5) s1(0) + (V3-V5)(s1(-2)+s1(+2)) + V5 (s1(-4)+s1(+4))]

  All 10 streams sit at EVEN column shifts {0,±2,±4}.  Three fp8 planes
  (stride 2064 = 129*16) are uploaded per core: hi=fp8(A*x), s1hi=fp8(A*s1)
  and a merged residual mu=fp8((A*x-hi)+(A*s1-s1hi)); A=1.0772 tunes the
  weight values onto the fp8 grid.  Per 512-col chunk: 8 DoubleRow passes:
  (hi|s1hi) pairs at dy in {0,±2,±4} plus mu zero-delta passes at {0,±2}
  with cross-talk-optimal bands (Wx+2*Ws)/3.  PSUM accumulates fp32, the
  A-scaling cancels (weights stored /A).  Host-sim rel err: 1.41e-2.

Engine budget: PE ~8*36 DR passes ~ 33us, DVE/ACT alternate PSUM->bf16
evictions, gpsimd does SWDGE stores, sync HWDGE loads (3 planes ride one
[k, 6192] row DMA per window).
"""
import numpy as np
import ml_dtypes
from contextlib import ExitStack

import concourse.bass as bass
import concourse.tile as tile
from concourse import bacc, mybir
from concourse.bass_utils import run_bass_kernel_spmd

N_CORES = 8
H = W = 4096
MAXD = 5
GRID_R, GRID_C = 4, 2
BR, BC = H // GRID_R, W // GRID_C      # 1024 x 2048
HALO_C = 4                             # max |dy| used by any stream
IN_ROWS = BR + 2 * MAXD                # 1034
PW = 2064                              # plane width (mult of 16) >= 2048+2*4
NPL = 3                                # hi | mu | s1hi
TW = NPL * PW                          # 6192 strip row bytes
NCOL = 512
NCHUNK = BC // NCOL                    # 4
M_OUT = 118
MPAD = 128                             # lhsT free per half (16B-aligned step)
ALPHA = 1.07725
N_WARM = 5
WINDOWS = []
_o = 0
while _o < BR:
    m = min(M_OUT, BR - _o)
    WINDOWS.append((_o, m, m + 2 * MAXD))
    _o += m

# pass list: (dy, kind) kind: 'xs' = (hi|s1hi) pair, 'mu' = mu zero-delta
PASSES = [(0, 'xs'), (0, 'mu'), (-2, 'xs'),
          (2, 'xs'), (-4, 'xs'), (4, 'xs')]
NPASS = len(PASSES)

_CACHE = {}
_F8 = ml_dtypes.float8_e4m3fn


def _band(vals: dict) -> np.ndarray:
    """[128, MPAD] band matrix W[k, m] = vals.get(k - m - MAXD)."""
    out = np.zeros((128, MPAD), dtype=np.float32)
    k = np.arange(128)[:, None]
    m = np.arange(MPAD)[None, :]
    d = k - m - MAXD
    for dd, v in vals.items():
        out += np.where(d == dd, np.float32(v), 0.0)
    return out


def _weights(distance_weights: np.ndarray) -> np.ndarray:
    """[128, NPASS*2*MPAD] fp8 stationary pairs, already divided by ALPHA."""
    wd = np.asarray(distance_weights, dtype=np.float64)
    w = {d: wd[d - 1] for d in range(1, MAXD + 1)}

    def band_x(dy):
        j = abs(dy)
        return {d: w[j + abs(d)] for d in range(-(MAXD - j), MAXD - j + 1)
                if j + abs(d) >= 1}

    def band_s(dy):
        j = abs(dy)
        if j == 0:
            return {d: w[1 + abs(d)] - (w[3 + abs(d)] if abs(d) <= 2 else 0.0)
                    + (w[5] if d == 0 else 0.0) for d in range(-4, 5)}
        if j == 2:
            return {d: w[3 + abs(d)] - (w[5] if d == 0 else 0.0)
                    for d in range(-2, 3)}
        return {0: w[5]}

    out = np.zeros((128, NPASS, 2, MPAD), dtype=np.float32)
    for p, (dy, kind) in enumerate(PASSES):
        bx, bs = band_x(dy), band_s(dy)
        if kind == 'xs':
            out[:, p, 0] = _band({d: v / ALPHA for d, v in bx.items()})
            out[:, p, 1] = _band({d: v / ALPHA for d, v in bs.items()})
        else:
            # mu carries xlo + s1lo + (xlo(-2)+xlo(+2)); variance-optimal
            # shared band (V0 + 2*Ws0 + 2*V2)/5 per tap.
            bx2 = band_x(2)
            taps = set(bx) | set(bs) | set(bx2)
            bmu = {d: (bx.get(d, 0.0) + 2 * bs.get(d, 0.0)
                       + 2 * bx2.get(d, 0.0)) / 5 / ALPHA for d in taps}
            out[:, p, 0] = _band(bmu)
            # second half: zeros (rhs reads mu twice with step-0 pair)
    flat = np.ascontiguousarray(out.reshape(128, NPASS * 2 * MPAD))
    return flat.astype(_F8)


def _build():
    dt8 = mybir.dt.float8e4
    dtb = mybir.dt.bfloat16
    dtf = mybir.dt.float32
    nc = bacc.Bacc("TRN2", target_bir_lowering=False, debug=False,
                   num_devices=N_CORES)
    x = nc.dram_tensor("x", [IN_ROWS, TW], dt8, kind="ExternalInput").ap()
    wts = nc.dram_tensor("w", [128, NPASS * 2 * MPAD], dt8,
                         kind="ExternalInput").ap()
    y = nc.dram_tensor("y", [BR, BC], dtb, kind="ExternalOutput").ap()

    with tile.TileContext(nc) as tc, ExitStack() as ctx:
        spool = ctx.enter_context(tc.tile_pool(name="strip", bufs=4))
        wpool = ctx.enter_context(tc.tile_pool(name="wts", bufs=1))
        opool = ctx.enter_context(tc.tile_pool(name="out", bufs=4))
        ppool = ctx.enter_context(tc.tile_pool(name="ps", bufs=8, space="PSUM"))

        wt = wpool.tile([128, NPASS * 2 * MPAD], dt8)
        nc.sync.dma_start(wt[:, :NPASS * MPAD], wts[:, :NPASS * MPAD])
        nc.sync.dma_start(wt[:, NPASS * MPAD:], wts[:, NPASS * MPAD:])
        CMID = TW // 2
        strips = {}
        o0, m0, k0 = WINDOWS[0]
        st = spool.tile([128, TW], dt8, tag="strip")
        nc.gpsimd.dma_start(st[:k0, :CMID], x[o0:o0 + k0, :CMID])
        nc.sync.dma_start(st[:k0, CMID:], x[o0:o0 + k0, CMID:])
        strips[0] = st

        # PE p-state warm-up on a zeroed bf16 scratch tile.
        zs = wpool.tile([128, M_OUT + NCOL], dtb, name="zs")
        nc.vector.memset(zs[:], 0)
        zp = ppool.tile([MPAD, NCOL], dtf, tag="ps", name="zp")
        for _ in range(N_WARM):
            nc.tensor.matmul(zp[:M_OUT, :], zs[:, :M_OUT], zs[:, M_OUT:],
                             start=True, stop=True)

        for wi, (out0, m, kdim) in enumerate(WINDOWS):
            if wi in strips:
                st = strips[wi]
            else:
                st = spool.tile([128, TW], dt8, tag="strip")
                nc.sync.dma_start(st[:kdim, :], x[out0:out0 + kdim, :])
            sap = st[:, :]
            ot = opool.tile([m, BC], dtb, tag="out")
            for cc in range(NCHUNK):
                c0 = cc * NCOL
                ps = ppool.tile([MPAD, NCOL], dtf, tag="ps")
                for i, (dy, kind) in enumerate(PASSES):
                    if kind == 'xs':
                        off = HALO_C + c0 + dy
                        rhs = bass.AP(sap.tensor, off,
                                      [[TW, 128], [2 * PW, 2], [1, NCOL]])
                    else:
                        off = PW + HALO_C + c0 + dy
                        rhs = bass.AP(sap.tensor, off,
                                      [[TW, 128], [0, 2], [1, NCOL]])
                    lhsT = wt[:, :].rearrange(
                        "k (p two mm) -> k p two mm",
                        p=NPASS, two=2)[:, i, :, :]
                    nc.tensor.matmul(
                        ps[:], lhsT, rhs,
                        start=(i == 0), stop=(i == NPASS - 1),
                        perf_mode=mybir.MatmulPerfMode.DoubleRow,
                    )
                # Evict fp32->bf16; alternate ACT/DVE to split the load.
                if cc % 2 == 1:
                    nc.vector.tensor_copy(ot[:, c0:c0 + NCOL], ps[:m, :])
                else:
                    nc.scalar.copy(ot[:, c0:c0 + NCOL], ps[:m, :])
                if wi == len(WINDOWS) - 1 and cc >= NCHUNK - 2:
                    nc.gpsimd.dma_start(y[out0:out0 + m, c0:c0 + NCOL],
                                        ot[:, c0:c0 + NCOL])
                elif cc % 2 == 1:
                    nc.gpsimd.dma_start(
                        y[out0:out0 + m, c0 - NCOL:c0 + NCOL],
                        ot[:, c0 - NCOL:c0 + NCOL])
    nc.compile()
    return nc


def _make_in_maps(grid_spikes: np.ndarray, distance_weights: np.ndarray):
    x = np.ascontiguousarray(grid_spikes, dtype=np.float32)
    assert x.shape == (H, W)
    w_flat = _weights(distance_weights)

    xb = x.astype(ml_dtypes.bfloat16).astype(np.float32)
    s1 = np.roll(xb, 1, axis=1) + np.roll(xb, -1, axis=1)
    axb = ALPHA * xb
    as1 = ALPHA * s1
    hi = axb.astype(_F8)
    s1hi = as1.astype(_F8)
    xlo = axb - hi.astype(np.float32)
    s1lo = as1 - s1hi.astype(np.float32)
    s2lo = np.roll(xlo, 2, axis=1) + np.roll(xlo, -2, axis=1)
    mu = (xlo + s1lo + s2lo).astype(_F8)

    # wrap-pad rows by MAXD, cols by HALO_C
    def pad(a):
        return np.pad(a, ((MAXD, MAXD), (HALO_C, HALO_C)), mode="wrap")
    hi_p, mu_p, s1_p = pad(hi), pad(mu), pad(s1hi)

    in_maps = []
    strip = np.zeros((IN_ROWS, TW), dtype=_F8)
    for c in range(N_CORES):
        rb, cb = divmod(c, GRID_C)
        r0, c0 = rb * BR, cb * BC
        s = np.zeros((IN_ROWS, TW), dtype=_F8)
        cw = BC + 2 * HALO_C
        s[:, 0:cw] = hi_p[r0:r0 + IN_ROWS, c0:c0 + cw]
        s[:, PW:PW + cw] = mu_p[r0:r0 + IN_ROWS, c0:c0 + cw]
        s[:, 2 * PW:2 * PW + cw] = s1_p[r0:r0 + IN_ROWS, c0:c0 + cw]
        in_maps.append({"x": s, "w": w_flat})
    return in_maps


def kernel(grid_spikes: np.ndarray, distance_weights: np.ndarray) -> np.ndarray:
    if "nc" not in _CACHE:
        _CACHE["nc"] = _build()
    nc = _CACHE["nc"]

    in_maps = _make_in_maps(grid_spikes, distance_weights)
    try:
        res = run_bass_kernel_spmd(nc, in_maps, list(range(N_CORES)))
    except Exception:
        res = run_bass_kernel_spmd(nc, in_maps, list(range(N_CORES)))
    out = np.empty((H, W), dtype=np.float32)
    for c in range(N_CORES):
        rb, cb = divmod(c, GRID_C)
        out[rb * BR:(rb + 1) * BR, cb * BC:(cb + 1) * BC] = \
            res.results[c]["y"].astype(np.float32)
    return out


# revision 17
# speedup vs baseline: 1.0205x; 1.0205x over previous
"""Trainium2 Bass kernel for nn_LocalConnectivity (diamond-ring circular stencil).

out[i,j] = sum_{d=1..5} w_d * sum_{|di|+|dj|=d} x[(i+di)%H, (j+dj)%W]

Strategy: 4x2 grid shard (1024x2048 block/core), all matmul work in fp8
DoubleRow mode (2 stacked 128-contraction streams per 256-cycle pass, 4x
the bf16 rate).  The dual-fp8 ISA requires pair strides ≡0 (mod 16B) and
even start offsets, which bans odd column shifts and same-plane ±j pairs.
Both are solved by a telescoped sigma-basis, computed on host:

  s1 = x(-1)+x(+1)  (host fp32 fold), then with V_j the vertical band of
  w_{j+|d|} taps:   sum_j V_j x(shift j)  ==
      [V0 x(0) + V2 (x(-2)+x(+2)) + V4 (x(-4)+x(+4))]            (x-family)
    + [(V1-V3+V5) s1(0) + (V3-V5)(s1(-2)+s1(+2)) + V5 (s1(-4)+s1(+4))]

  All 10 streams sit at EVEN column shifts {0,±2,±4}.  Three fp8 planes
  (stride 2064 = 129*16) are uploaded per core: hi=fp8(A*x), s1hi=fp8(A*s1)
  and a merged residual mu=fp8((A*x-hi)+(A*s1-s1hi)); A=1.0772 tunes the
  weight values onto the fp8 grid.  Per 512-col chunk: 8 DoubleRow passes:
  (hi|s1hi) pairs at dy in {0,±2,±4} plus mu zero-delta passes at {0,±2}
  with cross-talk-optimal bands (Wx+2*Ws)/3.  PSUM accumulates fp32, the
  A-scaling cancels (weights stored /A).  Host-sim rel err: 1.41e-2.

Engine budget: PE ~8*36 DR passes ~ 33us, DVE/ACT alternate PSUM->bf16
evictions, gpsimd does SWDGE stores, sync HWDGE loads (3 planes ride one
[k, 6192] row DMA per window).
"""
import numpy as np
import ml_dtypes
from contextlib import ExitStack

import concourse.bass as bass
import concourse.tile as tile
from concourse import bacc, mybir
from concourse.bass_utils import run_bass_kernel_spmd

N_CORES = 8
H = W = 4096
MAXD = 5
GRID_R, GRID_C = 4, 2
BR, BC = H // GRID_R, W // GRID_C      # 1024 x 2048
HALO_C = 4                             # max |dy| used by any stream
IN_ROWS = BR + 2 * MAXD                # 1034
# plane base offsets chosen so every DoubleRow pair has delta % 16 == 0 and
# even starts: xhi/s1hi at 0 mod 16, s2hi at +2, mu at -2 (14) mod 16
P_XHI, P_S1, P_S2, P_MU = 0, 2064, 4130, 6206
TW = 8272                              # strip row bytes (mult of 16)
NCOL = 512
NCHUNK = BC // NCOL                    # 4
M_OUT = 118
MPAD = 128                             # lhsT free per half (16B-aligned step)
ALPHA = 1.07725
N_WARM = 5
WINDOWS = []
_o = 0
while _o < BR:
    m = min(M_OUT, BR - _o)
    WINDOWS.append((_o, m, m + 2 * MAXD))
    _o += m

# pass list: pairs of (plane_base, dy, band) halves; all deltas % 16 == 0
PASSES = [
    ((P_XHI, 0, 'V0'), (P_S1, 0, 'A0')),
    ((P_S1, -2, 'A2'), (P_MU, 0, 'WMU')),
    ((P_S1, 2, 'A2'), (P_S2, 0, 'V2')),
    ((P_XHI, -4, 'V4'), (P_S1, -4, 'A4')),
    ((P_XHI, 4, 'V4'), (P_S1, 4, 'A4')),
]
NPASS = len(PASSES)

_CACHE = {}
_F8 = ml_dtypes.float8_e4m3fn


def _band(vals: dict) -> np.ndarray:
    """[128, MPAD] band matrix W[k, m] = vals.get(k - m - MAXD)."""
    out = np.zeros((128, MPAD), dtype=np.float32)
    k = np.arange(128)[:, None]
    m = np.arange(MPAD)[None, :]
    d = k - m - MAXD
    for dd, v in vals.items():
        out += np.where(d == dd, np.float32(v), 0.0)
    return out


def _weights(distance_weights: np.ndarray) -> np.ndarray:
    """[128, NPASS*2*MPAD] fp8 stationary pairs, already divided by ALPHA."""
    wd = np.asarray(distance_weights, dtype=np.float64)
    w = {d: wd[d - 1] for d in range(1, MAXD + 1)}

    def band_x(dy):
        j = abs(dy)
        return {d: w[j + abs(d)] for d in range(-(MAXD - j), MAXD - j + 1)
                if j + abs(d) >= 1}

    A0 = {d: w[1 + abs(d)] - (w[3 + abs(d)] if abs(d) <= 2 else 0.0)
          + (w[5] if d == 0 else 0.0) for d in range(-4, 5)}
    A2 = {d: w[3 + abs(d)] - (w[5] if d == 0 else 0.0) for d in range(-2, 3)}
    A4 = {0: w[5]}
    bx0, bx2 = band_x(0), band_x(2)
    # mu carries xlo + s1lo + s2lo; variance-optimal shared band
    # (V0 + 2*A0 + 2*V2)/5 per tap.
    taps = set(bx0) | set(A0) | set(bx2)
    WMU = {d: (bx0.get(d, 0.0) + 2 * A0.get(d, 0.0)
               + 2 * bx2.get(d, 0.0)) / 5 for d in taps}
    bands = {'V0': bx0, 'V2': bx2, 'V4': band_x(4),
             'A0': A0, 'A2': A2, 'A4': A4, 'WMU': WMU}
    out = np.zeros((128, NPASS, 2, MPAD), dtype=np.float32)
    for p, halves in enumerate(PASSES):
        for h, (_base, _dy, key) in enumerate(halves):
            out[:, p, h] = _band({d: v / ALPHA
                                  for d, v in bands[key].items()})
    flat = np.ascontiguousarray(out.reshape(128, NPASS * 2 * MPAD))
    return flat.astype(_F8)


def _build():
    dt8 = mybir.dt.float8e4
    dtb = mybir.dt.bfloat16
    dtf = mybir.dt.float32
    nc = bacc.Bacc("TRN2", target_bir_lowering=False, debug=False,
                   num_devices=N_CORES)
    x = nc.dram_tensor("x", [IN_ROWS, TW], dt8, kind="ExternalInput").ap()
    wts = nc.dram_tensor("w", [128, NPASS * 2 * MPAD], dt8,
                         kind="ExternalInput").ap()
    y = nc.dram_tensor("y", [BR, BC], dtb, kind="ExternalOutput").ap()

    with tile.TileContext(nc) as tc, ExitStack() as ctx:
        spool = ctx.enter_context(tc.tile_pool(name="strip", bufs=4))
        wpool = ctx.enter_context(tc.tile_pool(name="wts", bufs=1))
        opool = ctx.enter_context(tc.tile_pool(name="out", bufs=4))
        ppool = ctx.enter_context(tc.tile_pool(name="ps", bufs=8, space="PSUM"))

        wt = wpool.tile([128, NPASS * 2 * MPAD], dt8)
        nc.sync.dma_start(wt[:, :NPASS * MPAD], wts[:, :NPASS * MPAD])
        nc.sync.dma_start(wt[:, NPASS * MPAD:], wts[:, NPASS * MPAD:])
        CMID = TW // 2
        strips = {}
        o0, m0, k0 = WINDOWS[0]
        st = spool.tile([128, TW], dt8, tag="strip")
        nc.gpsimd.dma_start(st[:k0, :CMID], x[o0:o0 + k0, :CMID])
        nc.sync.dma_start(st[:k0, CMID:], x[o0:o0 + k0, CMID:])
        strips[0] = st

        # PE p-state warm-up on a zeroed bf16 scratch tile.
        zs = wpool.tile([128, M_OUT + NCOL], dtb, name="zs")
        nc.vector.memset(zs[:], 0)
        zp = ppool.tile([MPAD, NCOL], dtf, tag="ps", name="zp")
        for _ in range(N_WARM):
            nc.tensor.matmul(zp[:M_OUT, :], zs[:, :M_OUT], zs[:, M_OUT:],
                             start=True, stop=True)

        for wi, (out0, m, kdim) in enumerate(WINDOWS):
            if wi in strips:
                st = strips[wi]
            else:
                st = spool.tile([128, TW], dt8, tag="strip")
                nc.sync.dma_start(st[:kdim, :], x[out0:out0 + kdim, :])
            sap = st[:, :]
            ot = opool.tile([m, BC], dtb, tag="out")
            for cc in range(NCHUNK):
                c0 = cc * NCOL
                ps = ppool.tile([MPAD, NCOL], dtf, tag="ps")
                for i, ((bA, dA, _kA), (bB, dB, _kB)) in enumerate(PASSES):
                    offA = bA + HALO_C + c0 + dA
                    delta = (bB + HALO_C + c0 + dB) - offA
                    rhs = bass.AP(sap.tensor, offA,
                                  [[TW, 128], [delta, 2], [1, NCOL]])
                    lhsT = wt[:, :].rearrange(
                        "k (p two mm) -> k p two mm",
                        p=NPASS, two=2)[:, i, :, :]
                    nc.tensor.matmul(
                        ps[:], lhsT, rhs,
                        start=(i == 0), stop=(i == NPASS - 1),
                        perf_mode=mybir.MatmulPerfMode.DoubleRow,
                    )
                # Evict fp32->bf16; alternate ACT/DVE to split the load.
                if cc % 2 == 1:
                    nc.vector.tensor_copy(ot[:, c0:c0 + NCOL], ps[:m, :])
                else:
                    nc.scalar.copy(ot[:, c0:c0 + NCOL], ps[:m, :])
                if wi == len(WINDOWS) - 1 and cc >= NCHUNK - 2:
                    nc.gpsimd.dma_start(y[out0:out0 + m, c0:c0 + NCOL],
                                        ot[:, c0:c0 + NCOL])
                elif cc % 2 == 1:
                    nc.gpsimd.dma_start(
                        y[out0:out0 + m, c0 - NCOL:c0 + NCOL],
                        ot[:, c0 - NCOL:c0 + NCOL])
    nc.compile()
    return nc


def _make_in_maps(grid_spikes: np.ndarray, distance_weights: np.ndarray):
    x = np.ascontiguousarray(grid_spikes, dtype=np.float32)
    assert x.shape == (H, W)
    w_flat = _weights(distance_weights)

    xb = x.astype(ml_dtypes.bfloat16).astype(np.float32)
    s1 = np.roll(xb, 1, axis=1) + np.roll(xb, -1, axis=1)
    s2 = np.roll(xb, 2, axis=1) + np.roll(xb, -2, axis=1)
    axb = ALPHA * xb
    as1 = ALPHA * s1
    as2 = ALPHA * s2
    hi = axb.astype(_F8)
    s1hi = as1.astype(_F8)
    s2hi = as2.astype(_F8)
    xlo = axb - hi.astype(np.float32)
    s1lo = as1 - s1hi.astype(np.float32)
    s2lo = as2 - s2hi.astype(np.float32)
    mu = (xlo + s1lo + s2lo).astype(_F8)

    # wrap-pad rows by MAXD, cols by HALO_C
    def pad(a):
        return np.pad(a, ((MAXD, MAXD), (HALO_C, HALO_C)), mode="wrap")
    planes = [(P_XHI, pad(hi)), (P_S1, pad(s1hi)),
              (P_S2, pad(s2hi)), (P_MU, pad(mu))]

    in_maps = []
    for c in range(N_CORES):
        rb, cb = divmod(c, GRID_C)
        r0, c0 = rb * BR, cb * BC
        s = np.zeros((IN_ROWS, TW), dtype=_F8)
        cw = BC + 2 * HALO_C
        for base, ap in planes:
            s[:, base:base + cw] = ap[r0:r0 + IN_ROWS, c0:c0 + cw]
        in_maps.append({"x": s, "w": w_flat})
    return in_maps


def kernel(grid_spikes: np.ndarray, distance_weights: np.ndarray) -> np.ndarray:
    if "nc" not in _CACHE:
        _CACHE["nc"] = _build()
    nc = _CACHE["nc"]

    in_maps = _make_in_maps(grid_spikes, distance_weights)
    try:
        res = run_bass_kernel_spmd(nc, in_maps, list(range(N_CORES)))
    except Exception:
        res = run_bass_kernel_spmd(nc, in_maps, list(range(N_CORES)))
    out = np.empty((H, W), dtype=np.float32)
    for c in range(N_CORES):
        rb, cb = divmod(c, GRID_C)
        out[rb * BR:(rb + 1) * BR, cb * BC:(cb + 1) * BC] = \
            res.results[c]["y"].astype(np.float32)
    return out
